# revision 29
# baseline (speedup 1.0000x reference)
"""DeepJ (TimeAxis + NoteAxis LSTM) Trainium2 kernel.

Data-parallel over 8 NeuronCores: batch 1024 -> 128 per core.

Layout strategy ("everything transposed"):
  activations live as [units, rows] tiles with rows = (note, batch) on the
  free dimension; weights are the stationary (lhsT) matmul operands.  The
  NoteAxis recurrence then needs no per-step transposes: each step's gate
  matmuls consume the previous step's h tiles directly as rhs.

All matmuls run in bfloat16 (PE full rate).  The NoteAxis input projection
(lnf @ nf + lsh @ shifted, bias included) is materialized ahead of the
recurrence into one note-major SBUF tile G0all (col = n*512 + q*128 + b)
by a TimeAxis-style batched pass, then injected into PSUM per step with a
single identity-stationary 512-col matmul; the per-step loop only runs the
three recurrent weight matrices (lhh0, lih1, lhh1) plus the injects.

PSUM accumulation rule (hardware-verified): a start=True matmul clears the
has_written bits of its ENTIRE PSUM bank, so each bank gets exactly one
start=True per accumulation round (the first matmul into it); all other
matmuls use start=False - first touch of a region overwrites via the
cleared bits, later touches accumulate.

The PE executes its queue in order, so the L1 recurrent matmul (lhh1,
which waits on h1) is emitted in na_step AFTER lhh0 - the critical
L0->L0 recurrence never queues behind the L1 chain.
"""

import sys

for _p in ("/opt/trn_rl_repo",):
    if _p not in sys.path:
        sys.path.insert(0, _p)

import numpy as np

# ---- model constants -------------------------------------------------------
N_CORES = 8
B_TOT = 1024
B = B_TOT // N_CORES          # 128 rows per core
NN = 48                       # notes
OCT = 12
R = NN * B                    # 6144 rows, ordered (note, batch)
NBLK = 12                     # row blocks of 512 for the feed-forward stages
BLK = 512

_PROGRAM_CACHE = {}


def _build_program():
    import concourse.tile as tile
    from concourse import bacc, mybir



    f32 = mybir.dt.float32
    bf16 = mybir.dt.bfloat16

    nc = bacc.Bacc(
        "TRN2", target_bir_lowering=False, debug=False, num_devices=N_CORES
    )

    def param(name, shape, dtype=f32):
        return nc.declare_dram_parameter(name, list(shape), dtype, isOutput=False)

    P = {}
    # per-core activations / gathered inputs
    P["im2colT"] = param("im2colT", [75, R], bf16)  # conv patches, (c*25+s, (n,b))
    P["beat_bc"] = param("beat_bc", [16, R], bf16)  # beat_in^T broadcast over n
    P["e48"] = param("e48", [48, R], bf16)          # one-hot(n) broadcast over b
    P["note0T"] = param("note0T", [48, B], bf16)    # note_input[:,:,0]^T
    P["shiftedT"] = param("shiftedT", [4, R], bf16)  # row 3 = ones (bias)
    P["outb_bc"] = param("outb_bc", [128, 3])
    # weights (replicated on every core)
    P["w0comb"] = param("w0comb", [108, 768], bf16)  # folded TA-L0 lhsT
    P["lvic"] = param("lvic", [75, 32], bf16)        # conv lhsT
    P["vicb"] = param("vicb", [32, 1])
    P["lsel"] = param("lsel", [48, 12], bf16)        # chord selection lhsT
    P["w1a"] = param("w1a", [128, 768], bf16)        # TA-L1 lhsT rows 0-127
    P["w1b"] = param("w1b", [128, 768], bf16)        # TA-L1 lhsT rows 128-255
    P["b1cols"] = param("b1cols", [128, 6])        # TA-L1 bias, col per chunk
    P["lnf0"] = param("lnf0", [128, 512], bf16)    # NA-L0 Wih (nf) lhsT
    P["lnf1"] = param("lnf1", [128, 512], bf16)
    P["lsh"] = param("lsh", [4, 512], bf16)        # NA-L0 Wih shifted+bias lhsT
    P["lhh0"] = param("lhh0", [128, 512], bf16)    # NA-L0 Whh lhsT
    P["lih1"] = param("lih1", [128, 512], bf16)    # NA-L1 Wih lhsT
    P["lhh1"] = param("lhh1", [128, 512], bf16)    # NA-L1 Whh lhsT
    P["bias_bc"] = param("bias_bc", [128, 512], bf16)  # NA-L1 bias bcast (g 2x)
    P["ident"] = param("ident", [128, 128], bf16)  # identity (inject lhsT)
    P["outWT"] = param("outWT", [128, 3], bf16)
    P["yout"] = nc.declare_dram_parameter("y", [B, NN * 3], f32, isOutput=True)
    import os as _os
    if _os.environ.get("DEEPJ_DEBUG"):
        for nm, shp, dt in [("d_xt", [108, R], bf16), ("d_h0a", [128, R], bf16),
                            ("d_h0b", [128, R], bf16), ("d_nfa", [128, R], bf16),
                            ("d_nfb", [128, R], bf16), ("d_h1", [128, R], bf16),
                            ("d_g0", [128, 512 * NN], f32),
                            ("d_s0", [128, 512 * NN], f32),
                            ("d_g1", [128, 512 * NN], f32),
                            ("d_h0r", [128, 128 * NN], f32)]:
            P[nm] = nc.declare_dram_parameter(nm, shp, dt, isOutput=True)

    with tile.TileContext(nc) as tc:
        _emit(nc, tc, mybir, P)
    nc.compile()
    return nc


def _emit(nc, tc, mybir, P):
    import os as _os
    from contextlib import ExitStack

    f32 = mybir.dt.float32
    bf16 = mybir.dt.bfloat16
    AF = mybir.ActivationFunctionType
    Alu = mybir.AluOpType

    with ExitStack() as top:
        wpool = top.enter_context(tc.tile_pool(name="weights", bufs=1))
        persist = top.enter_context(tc.tile_pool(name="persist", bufs=1))
        scr = top.enter_context(tc.tile_pool(name="scr", bufs=1))
        nascr = top.enter_context(tc.tile_pool(name="nascr", bufs=2))
        h0ring = top.enter_context(tc.tile_pool(name="h0ring", bufs=3))
        cpool = top.enter_context(tc.tile_pool(name="cstate", bufs=2))
        # PSUM budget (8 banks): pio [128,1024] x2 bufs = 4, ps0 1, ps1 1,
        # pout (g0/conv/chord/outproj) [128,512] x2 bufs = 2.
        pta = top.enter_context(tc.tile_pool(name="pta", bufs=2, space="PSUM"))
        pna0 = top.enter_context(tc.tile_pool(name="pna0", bufs=1, space="PSUM"))
        pna1 = top.enter_context(tc.tile_pool(name="pna1", bufs=1, space="PSUM"))
        pout = top.enter_context(tc.tile_pool(name="pout", bufs=2, space="PSUM"))

        def wload(name, shape, dtype=f32):
            t = wpool.tile(list(shape), dtype, tag=name, name=name)
            nc.sync.dma_start(t[:], P[name][:])
            return t

        # persistent activations (declare first; DMA issue order below is
        # the startup-critical-path order)
        xt = persist.tile([108, R], bf16, tag="xt")
        h0T = [persist.tile([128, R], bf16, tag=f"h0T{i}", name=f"h0T{i}")
               for i in range(2)]
        nfT = [persist.tile([128, R], bf16, tag=f"nfT{i}", name=f"nfT{i}")
               for i in range(2)]
        # NA-L0 input projection, note-major: col = n*512 + q*128 + b
        G0all = persist.tile([128, 4 * R], bf16, tag="G0all")
        h1All = persist.tile([128, R], bf16, tag="h1All")
        imT = persist.tile([75, R], bf16, tag="imT")

        # startup-critical loads first: conv + chord + TA-L0 inputs
        nc.sync.dma_start(imT[:, 0:2 * BLK], P["im2colT"][:, 0:2 * BLK])
        lvic_t = wload("lvic", [75, 32], bf16)
        vicb_t = wload("vicb", [32, 1])
        lsel_t = wload("lsel", [48, 12], bf16)
        w0comb_t = wload("w0comb", [108, 768], bf16)
        n0_t = scr.tile([48, B], bf16, tag="note0T")
        nc.sync.dma_start(n0_t[:], P["note0T"][:])
        nc.sync.dma_start(xt[32:48, :], P["beat_bc"][:])
        nc.sync.dma_start(xt[48:96, :], P["e48"][:])
        nc.sync.dma_start(imT[:, 2 * BLK:], P["im2colT"][:, 2 * BLK:])
        w1a_t = wload("w1a", [128, 768], bf16)
        w1b_t = wload("w1b", [128, 768], bf16)
        b1cols_t = wload("b1cols", [128, 6])
        ident_t = wload("ident", [128, 128], bf16)
        lnf0_t = wload("lnf0", [128, 512], bf16)
        lnf1_t = wload("lnf1", [128, 512], bf16)
        lsh_t = wload("lsh", [4, 512], bf16)
        shT_t = wload("shiftedT", [4, R], bf16)
        lhh0_t = wload("lhh0", [128, 512], bf16)
        lih1_t = wload("lih1", [128, 512], bf16)
        lhh1_t = wload("lhh1", [128, 512], bf16)
        bias_bc_t = wload("bias_bc", [128, 512], bf16)
        outWT_t = wload("outWT", [128, 3], bf16)
        outb_t = wload("outb_bc", [128, 3])

        # ---- one-time XT rows: chord ------------------------------------
        cps = pout.tile([32, BLK], f32, tag="g0ps", name="cps")
        nc.tensor.matmul(cps[0:12, 0:B], lsel_t[:], n0_t[:])
        chT = scr.tile([12, B], bf16, tag="chT")
        nc.vector.tensor_copy(chT[:], cps[0:12, 0:B])
        for n in range(NN):
            nc.sync.dma_start(xt[96:108, n * B:(n + 1) * B], chT[:])

        # ---- TA block emitters (chunked for fine-grained weaving) -------
        def ta_conv(blk):
            sl = slice(blk * BLK, (blk + 1) * BLK)
            vps = pout.tile([32, BLK], f32, tag="g0ps", name="vps")
            nc.tensor.matmul(vps[:], lvic_t[:], imT[:, sl])
            nc.scalar.activation(xt[0:32, sl], vps[:], AF.Tanh,
                                 bias=vicb_t[:, 0:1])

        def ta_l0_chunks(blk):
            """TA layer 0 for one 512-row block as 4 weavable chunks."""
            sl = slice(blk * BLK, (blk + 1) * BLK)
            st = {}

            # All activation / DVE / GPSIMD filler ops are emitted in
            # ~512-col pieces: the static schedule interleaves them with the
            # NA chain ops, and a chain op never waits more than one small
            # piece for its engine.
            def act2(out, in_, func, n=2):
                w = 2 * BLK // n
                for j in range(n):
                    js = slice(j * w, (j + 1) * w)
                    nc.scalar.activation(out[:, js], in_[:, js], func)

            def c0():
                x = xt[:, sl]
                io0 = pta.tile([128, 2 * BLK], f32, tag="pio", name="io0")
                nc.tensor.matmul(io0[:, 0:BLK], w0comb_t[:, 0:128], x)
                nc.tensor.matmul(io0[:, BLK:2 * BLK], w0comb_t[:, 512:640], x)
                s0 = scr.tile([128, 2 * BLK], f32, tag="sio0")
                act2(s0, io0, AF.Sigmoid)
                st["s0"] = s0

            def c1():
                x = xt[:, sl]
                io1 = pta.tile([128, 2 * BLK], f32, tag="pio", name="io1")
                nc.tensor.matmul(io1[:, 0:BLK], w0comb_t[:, 128:256], x)
                nc.tensor.matmul(io1[:, BLK:2 * BLK], w0comb_t[:, 640:768], x)
                s1 = scr.tile([128, 2 * BLK], f32, tag="sio1")
                act2(s1, io1, AF.Sigmoid)
                st["s1"] = s1

            def c2():
                x = xt[:, sl]
                gg = pta.tile([128, 2 * BLK], f32, tag="pio", name="gg")
                nc.tensor.matmul(gg[:, 0:BLK], w0comb_t[:, 256:384], x)
                nc.tensor.matmul(gg[:, BLK:2 * BLK], w0comb_t[:, 384:512], x)
                tg = scr.tile([128, 2 * BLK], f32, tag="tg")
                act2(tg, gg, AF.Tanh)
                cc = scr.tile([128, 2 * BLK], f32, tag="c2")
                for j in range(4):
                    js = slice(j * 256, (j + 1) * 256)
                    ss = st["s0"] if j < 2 else st["s1"]
                    nc.gpsimd.tensor_tensor(cc[:, js], ss[:, (j % 2) * 256:
                                            (j % 2) * 256 + 256],
                                            tg[:, js], Alu.mult)
                st["cc"] = cc

            def c3():
                tc2 = scr.tile([128, 2 * BLK], f32, tag="tc2")
                act2(tc2, st["cc"], AF.Tanh)
                for j in range(2):
                    js = slice(j * 256, (j + 1) * 256)
                    nc.vector.tensor_tensor(h0T[0][:, sl][:, js],
                                            st["s0"][:, BLK:2 * BLK][:, js],
                                            tc2[:, 0:BLK][:, js], Alu.mult)
                    nc.vector.tensor_tensor(h0T[1][:, sl][:, js],
                                            st["s1"][:, BLK:2 * BLK][:, js],
                                            tc2[:, BLK:2 * BLK][:, js],
                                            Alu.mult)

            return [c0, c1, c2, c3]

        def _l1_bank(sl, ps, cols, q):
            """One TA-L1 psum bank: w1a/w1b for u-chunk q (bias rides on
            the activation's per-partition bias operand, not a matmul)."""
            qs = slice(q * 128, (q + 1) * 128)
            nc.tensor.matmul(ps[:, cols], w1a_t[:, qs], h0T[0][:, sl],
                             start=True, stop=False, skip_group_check=True)
            nc.tensor.matmul(ps[:, cols], w1b_t[:, qs], h0T[1][:, sl],
                             start=False, stop=True, skip_group_check=True)

        def ta_l1_chunks(blk):
            """TA layer 1 for one 512-row block as 4 weavable chunks."""
            sl = slice(blk * BLK, (blk + 1) * BLK)
            st = {}

            def c0():
                io0 = pta.tile([128, 2 * BLK], f32, tag="pio", name="l1io0")
                _l1_bank(sl, io0, slice(0, BLK), 0)          # i half0
                _l1_bank(sl, io0, slice(BLK, 2 * BLK), 4)    # o half0
                s0 = scr.tile([128, 2 * BLK], f32, tag="sio0", name="bsio0")
                nc.scalar.activation(s0[:, 0:BLK], io0[:, 0:BLK], AF.Sigmoid,
                                     bias=b1cols_t[:, 0:1])
                nc.scalar.activation(s0[:, BLK:2 * BLK], io0[:, BLK:2 * BLK],
                                     AF.Sigmoid, bias=b1cols_t[:, 4:5])
                st["s0"] = s0

            def c1():
                io1 = pta.tile([128, 2 * BLK], f32, tag="pio", name="l1io1")
                _l1_bank(sl, io1, slice(0, BLK), 1)          # i half1
                _l1_bank(sl, io1, slice(BLK, 2 * BLK), 5)    # o half1
                s1 = scr.tile([128, 2 * BLK], f32, tag="sio1", name="bsio1")
                nc.scalar.activation(s1[:, 0:BLK], io1[:, 0:BLK], AF.Sigmoid,
                                     bias=b1cols_t[:, 1:2])
                nc.scalar.activation(s1[:, BLK:2 * BLK], io1[:, BLK:2 * BLK],
                                     AF.Sigmoid, bias=b1cols_t[:, 5:6])
                st["s1"] = s1

            def c2():
                gg = pta.tile([128, 2 * BLK], f32, tag="pio", name="l1gg")
                _l1_bank(sl, gg, slice(0, BLK), 2)           # g half0
                _l1_bank(sl, gg, slice(BLK, 2 * BLK), 3)     # g half1
                tg = scr.tile([128, 2 * BLK], f32, tag="tg", name="btg")
                nc.scalar.activation(tg[:, 0:BLK], gg[:, 0:BLK], AF.Tanh,
                                     bias=b1cols_t[:, 2:3])
                nc.scalar.activation(tg[:, BLK:2 * BLK], gg[:, BLK:2 * BLK],
                                     AF.Tanh, bias=b1cols_t[:, 3:4])
                cc = scr.tile([128, 2 * BLK], f32, tag="c2", name="bc2")
                for j in range(4):
                    js = slice(j * 256, (j + 1) * 256)
                    ss = st["s0"] if j < 2 else st["s1"]
                    nc.gpsimd.tensor_tensor(cc[:, js], ss[:, (j % 2) * 256:
                                            (j % 2) * 256 + 256],
                                            tg[:, js], Alu.mult)
                st["cc"] = cc

            def c3():
                tc2 = scr.tile([128, 2 * BLK], f32, tag="tc2", name="btc2")
                for j in range(2):
                    js = slice(j * BLK, (j + 1) * BLK)
                    nc.scalar.activation(tc2[:, js], st["cc"][:, js], AF.Tanh)
                for j in range(2):
                    js = slice(j * 256, (j + 1) * 256)
                    nc.vector.tensor_tensor(nfT[0][:, sl][:, js],
                                            st["s0"][:, BLK:2 * BLK][:, js],
                                            tc2[:, 0:BLK][:, js], Alu.mult)
                    nc.vector.tensor_tensor(nfT[1][:, sl][:, js],
                                            st["s1"][:, BLK:2 * BLK][:, js],
                                            tc2[:, BLK:2 * BLK][:, js],
                                            Alu.mult)

            return [c0, c1, c2, c3]

        G0v = G0all[:].rearrange("p (n q b) -> p n q b", q=4, b=B)

        def g0_chunk(blk, q):
            """NA-L0 input projection for gate q over 4 notes (512 rows)."""
            sl = slice(blk * BLK, (blk + 1) * BLK)
            qs = slice(q * 128, (q + 1) * 128)
            gps = pout.tile([128, BLK], f32, tag="g0ps", name="g0ps")
            nc.tensor.matmul(gps[:], lnf0_t[:, qs], nfT[0][:, sl],
                             start=True, stop=False, skip_group_check=True)
            nc.tensor.matmul(gps[:], lnf1_t[:, qs], nfT[1][:, sl],
                             start=False, stop=False, skip_group_check=True)
            nc.tensor.matmul(gps[:], lsh_t[:, qs], shT_t[:, sl],
                             start=False, stop=True, skip_group_check=True)
            # GPSIMD cannot read PSUM, so this cast stays on the DVE
            # (two 2-note pieces to bound NA-chain queue delay)
            g3 = gps[:].rearrange("p (n b) -> p n b", b=B)
            for j in range(2):
                nc.vector.tensor_copy(
                    G0v[:, 4 * blk + 2 * j:4 * blk + 2 * j + 2, q, :],
                    g3[:, 2 * j:2 * j + 2, :])

        # ---- NoteAxis pair-emission pipeline ---------------------------
        # pair_emit(k) emits L0(k) and L1(k-1) with the ops interleaved so
        # the L0->L0 recurrence (lhh0 -> sigma0 -> DVE -> tanh -> h0r) owns
        # the front of every engine queue; L1(k-1) trails.  ps1(k-1) gets
        # lih1 BEFORE lhh1 (h1(k-2) is long ready), so the PE never stalls
        # on a fresh h1.  t2 = f*c_prev runs on GPSIMD, off the DVE path.
        ps0_state = {}
        ps1_state = {}
        h0_ring = {}
        c_prev = [None, None]
        pso_box = [None]

        def na_open0(k):
            ps0 = pna0.tile([128, 512], f32, tag="na0", name="ps0")
            nc.tensor.matmul(ps0[:], ident_t[:],
                             G0all[:, k * 512:(k + 1) * 512],
                             start=True, stop=(k == 0),
                             skip_group_check=True)
            ps0_state[k] = ps0

        def na_open1(k):
            """ps1 bias inject for L1(k); lih1/lhh1 accumulate next pair."""
            ps1 = pna1.tile([128, 512], f32, tag="na1", name="ps1")
            nc.tensor.matmul(ps1[:, 0:512], ident_t[:], bias_bc_t[:],
                             start=True, stop=False, skip_group_check=True)
            ps1_state[k] = ps1

        def _nl_front(s, li):
            """sigmoid -> gt/t1/(t2)/c chain for layer li; returns c_new.

            The cell state is stored HALVED (c' = c/2): with the host-side
            sigma-trick (g rows doubled), tanh(g) = 2*sigmoid(2g)-1, so
              c' = si*(sg - 0.5) + sf*c'_prev
            needs only a 1-ALU tensor_scalar, and tanh(c) = tanh(2*c') rides
            the activation's free scale operand downstream.
            """
            tag = f"L{li}"
            si, sf, sg, so = (s[:, 128 * j:128 * (j + 1)] for j in range(4))
            gt = nascr.tile([128, 128], f32, tag=f"{tag}gt")
            nc.vector.tensor_scalar(gt[:], sg, -0.5, None, Alu.add)
            # (1-ALU op; the former 2*s-1 needed mult+add at half DVE rate)
            c_new = cpool.tile([128, 128], f32, tag=f"{tag}c")
            if c_prev[li] is None:
                nc.vector.tensor_tensor(c_new[:], si, gt[:], Alu.mult)
            else:
                t2 = nascr.tile([128, 128], f32, tag=f"{tag}t2")
                nc.gpsimd.tensor_tensor(t2[:], sf, c_prev[li][:], Alu.mult)
                t1 = nascr.tile([128, 128], f32, tag=f"{tag}t1")
                nc.vector.tensor_tensor(t1[:], si, gt[:], Alu.mult)
                # L0's add is on the h0r critical path (fast DVE); L1's is
                # not, so it rides the idler GPSIMD engine.
                eng = nc.vector if li == 0 else nc.gpsimd
                eng.tensor_tensor(c_new[:], t1[:], t2[:], Alu.add)
            c_prev[li] = c_new
            return c_new

        def pair_emit(k):
            # The NA recurrence is the serial critical path of the whole
            # kernel: give its ops priority 0 so the list scheduler runs
            # them the moment their deps resolve, with TA/G0 work as the
            # natural filler on every engine.
            with tc.high_priority():
                _pair_emit(k)

        def _pair_emit(k):
            has0 = k < NN
            has1 = k >= 1
            # PE front: the two matmul groups gated on h0r(k-1)
            if has0:
                ps0 = ps0_state.pop(k)
                if k > 0:
                    h0p = h0_ring[k - 1]
                    for q in range(4):
                        qs = slice(q * 128, (q + 1) * 128)
                        nc.tensor.matmul(ps0[:, qs], lhh0_t[:, qs], h0p[:],
                                         start=False, stop=(q == 3),
                                         skip_group_check=True)
            if has1:
                ps1 = ps1_state.pop(k - 1)
                h0p = h0_ring.pop(k - 1)
                for q in range(4):
                    qs = slice(q * 128, (q + 1) * 128)
                    nc.tensor.matmul(ps1[:, qs], lih1_t[:, qs], h0p[:],
                                     start=False, stop=(q == 3 and k < 2),
                                     skip_group_check=True)
                if k >= 2:
                    pns = slice((k - 2) * B, (k - 1) * B)
                    for q in range(4):
                        qs = slice(q * 128, (q + 1) * 128)
                        nc.tensor.matmul(ps1[:, qs], lhh1_t[:, qs],
                                         h1All[:, pns], start=False,
                                         stop=(q == 3), skip_group_check=True)
            # scalar front: sigma0 before sigma1
            s0 = s1 = None
            if has0:
                s0 = nascr.tile([128, 512], f32, tag="L0s")
                nc.scalar.activation(s0[:], ps0[:], AF.Sigmoid)
            if has1:
                s1 = nascr.tile([128, 512], f32, tag="L1s")
                nc.scalar.activation(s1[:], ps1[:], AF.Sigmoid)
            # DVE chains: L0 first, then L1
            if has0:
                c0n = _nl_front(s0[:], 0)
                tc0 = nascr.tile([128, 128], f32, tag="L0tc")
                nc.scalar.activation(tc0[:], c0n[:], AF.Tanh, scale=2.0)
                h0r = h0ring.tile([128, B], bf16, tag="h0r", name="h0r")
                h0_ring[k] = h0r
                nc.vector.tensor_tensor(h0r[:], s0[:, 384:512], tc0[:],
                                        Alu.mult)
            if has1:
                c1n = _nl_front(s1[:], 1)
                tc1 = nascr.tile([128, 128], f32, tag="L1tc")
                nc.scalar.activation(tc1[:], c1n[:], AF.Tanh, scale=2.0)
                nc.vector.tensor_tensor(h1All[:, (k - 1) * B:k * B],
                                        s1[:, 384:512], tc1[:], Alu.mult)
            # openers
            if has0 and k + 1 < NN:
                na_open0(k + 1)
            if has0:
                na_open1(k)

        def outproj(n):
            nc.tensor.matmul(pso_box[0][:, 3 * n:3 * n + 3],
                             h1All[:, n * B:(n + 1) * B], outWT_t[:],
                             skip_group_check=True)

        # ---- interleaved pipeline: fine-grained TA/NA weave ------------
        # G0 chunks for blk b are emitted at the END of blk b (right after
        # l1c3 writes nfT(b)), so every pair woven through blk b+1 reads
        # fully-written G0 data.  Pairs run one block earlier than the
        # classic 2-block lag: pairs 0-2 right after blk 0's G0, then
        # 4b-5..4b-2 through blk b, leaving only 43-47 for the tail.
        for blk in range(NBLK):
            slots = [lambda b=blk: ta_conv(b)]
            slots += ta_l0_chunks(blk)
            slots += ta_l1_chunks(blk)
            slots += [lambda b=blk, q=q: g0_chunk(b, q) for q in range(4)]
            if blk >= 2:
                pairs = [4 * (blk - 2) + 3 + j for j in range(4)]
                # spread the 4 pair_emits across the 13 slots
                order = []
                pi = 0
                for i, sfn in enumerate(slots):
                    order.append(sfn)
                    if i in (2, 5, 8, 12) and pi < 4:
                        order.append(lambda p=pairs[pi]: pair_emit(p))
                        pi += 1
                for fn in order:
                    fn()
            else:
                for fn in slots:
                    fn()
            if blk == 0:
                na_open0(0)
            if blk == 1:
                pair_emit(0)
                pair_emit(1)
                pair_emit(2)
        pso_box[0] = pout.tile([128, BLK], f32, tag="g0ps", name="pso")
        pso = pso_box[0]
        for j in range(6):
            pair_emit(43 + j)
            for n in range(8 * j, min(8 * j + 8, 43)):  # backlog notes 0..42
                outproj(n)
        for n in range(43, NN):
            outproj(n)

        # ---- output sigmoid + store ------------------------------------
        out_sb = scr.tile([128, NN * 3], f32, tag="osb")
        ps3d = pso[:, 0:NN * 3].rearrange("p (n c) -> p n c", c=3)
        o3d = out_sb[:].rearrange("p (n c) -> p n c", c=3)
        nc.scalar.activation(o3d[:, :, 0], ps3d[:, :, 0], AF.Sigmoid,
                             bias=outb_t[:, 0:1])
        nc.scalar.activation(o3d[:, :, 1], ps3d[:, :, 1], AF.Sigmoid,
                             bias=outb_t[:, 1:2])
        nc.scalar.activation(o3d[:, :, 2], ps3d[:, :, 2], AF.Identity,
                             bias=outb_t[:, 2:3])
        nc.sync.dma_start(P["yout"][:], out_sb[:])


# --------------------------------------------------------------------------
# host side
# --------------------------------------------------------------------------

def _host_prep_weights(inp):
    import ml_dtypes

    f32 = np.float32
    bf16 = ml_dtypes.bfloat16

    W0 = np.asarray(inp["ta_Wih0"], f32)          # [1024, 73]
    sel = np.r_[0:256, 512:768, 768:1024]
    W0s = W0[sel]                                  # [768, 73] rows i,g,o
    b0s = (np.asarray(inp["ta_bih0"], f32) + np.asarray(inp["ta_bhh0"], f32))[sel]

    n = np.arange(NN)
    const_feat = np.zeros((13, NN), f32)
    const_feat[0] = n / NN
    const_feat[1 + (n % OCT), n] = 1.0

    beat_W = np.asarray(inp["beat_W"], f32)        # [16, 16]
    beat_b = np.asarray(inp["beat_b"], f32)
    gn = (W0s[:, 0:13] @ const_feat
          + (b0s + W0s[:, 13:29] @ beat_b)[:, None])        # [768, 48]
    Wbeat = W0s[:, 13:29] @ beat_W                 # [768, 16]
    Wvic = W0s[:, 29:61]                           # [768, 32]
    Wchord = W0s[:, 61:73]                         # [768, 12]
    w0comb = np.concatenate(
        [Wvic.T, Wbeat.T, gn.T, Wchord.T], axis=0
    ).astype(bf16)                                 # [108, 768]

    vic_W = np.asarray(inp["vic_W"], f32)          # [32, 3, 25]
    lvic = vic_W.reshape(32, 75).T.copy().astype(bf16)  # [75, 32] rows (c*25+s)
    vicb = np.asarray(inp["vic_b"], f32).reshape(32, 1)

    lsel = np.zeros((48, 12), f32)
    lsel[np.arange(48), np.arange(48) // 4] = 0.25
    lsel = lsel.astype(bf16)

    W1 = np.asarray(inp["ta_Wih1"], f32)[sel]      # [768, 256]
    b1s = (np.asarray(inp["ta_bih1"], f32) + np.asarray(inp["ta_bhh1"], f32))[sel]
    w1T = W1.T.astype(bf16)                        # [256, 768]
    b1cols = np.ascontiguousarray(b1s.reshape(6, 128).T)   # [128, 6] f32

    # sigma-trick: tanh(g) = 2*sigmoid(2g)-1, so double every g-gate row
    # (cols 256:384 of the transposed layouts) including the bias.
    def dbl_g(wT):
        wT = wT.copy()
        wT[:, 256:384] *= 2.0
        return wT

    naW0 = np.asarray(inp["na_Wih0"], f32)         # [512, 259]
    lnf = dbl_g(naW0[:, 0:256].T).astype(bf16)     # [256, 512]
    nb0 = (np.asarray(inp["na_bih0"], f32) + np.asarray(inp["na_bhh0"], f32))
    lsh = np.concatenate([naW0[:, 256:259].T, nb0[None, :]], axis=0)
    lsh = dbl_g(lsh).astype(bf16)                  # [4, 512]
    lhh0 = dbl_g(np.asarray(inp["na_Whh0"], f32).T).astype(bf16)
    lih1 = dbl_g(np.asarray(inp["na_Wih1"], f32).T).astype(bf16)
    lhh1 = dbl_g(np.asarray(inp["na_Whh1"], f32).T).astype(bf16)
    nb1 = (np.asarray(inp["na_bih1"], f32) + np.asarray(inp["na_bhh1"], f32))
    nb1d = nb1.copy()
    nb1d[256:384] *= 2.0                           # g-gate bias doubled
    # bias_bc[u, q*128 + b] = nb1d[q*128 + u], broadcast over batch b
    bias_bc = np.ascontiguousarray(
        np.broadcast_to(nb1d.reshape(4, 128).T[:, :, None], (128, 4, 128))
    ).reshape(128, 512).astype(bf16)

    outWT = np.asarray(inp["out_W"], f32).T.astype(bf16)     # [128, 3]
    outb_bc = np.broadcast_to(
        np.asarray(inp["out_b"], f32), (128, 3)
    ).copy()

    return {
        "w0comb": w0comb, "lvic": lvic, "vicb": vicb, "lsel": lsel,
        "w1a": w1T[0:128].copy(), "w1b": w1T[128:256].copy(), "b1cols": b1cols,
        "lnf0": lnf[0:128].copy(), "lnf1": lnf[128:256].copy(),
        "lsh": lsh, "lhh0": lhh0,
        "lih1": lih1, "lhh1": lhh1, "bias_bc": bias_bc,
        "ident": np.eye(128, dtype=bf16),
        "outWT": outWT, "outb_bc": outb_bc,
    }


def _host_prep_core(note, beat, cond):
    """Per-core input gathering (indexing only). note [B,48,3] etc."""
    import ml_dtypes

    f32 = np.float32
    bf16 = ml_dtypes.bfloat16
    pn = np.zeros((B, 72, 3), f32)
    pn[:, 12:60, :] = note
    # im2colT[(c*25+s), (n, b)] = pn[b, n+s, c]
    win = np.stack([pn[:, s:s + 48, :] for s in range(25)], axis=0)  # [25,B,48,3]
    im2colT = np.ascontiguousarray(win.transpose(3, 0, 2, 1)).reshape(75, R)

    beat_bc = np.ascontiguousarray(
        np.broadcast_to(beat.T[:, None, :], (16, NN, B))
    ).reshape(16, R)
    e48 = np.repeat(np.eye(48, dtype=bf16), B, axis=1)       # [48, R]
    note0T = np.ascontiguousarray(note[:, :, 0].T)           # [48, B]

    sh = np.zeros((B, NN, 3), f32)
    sh[:, 1:, :] = cond[:, :-1, :]
    shiftedT = np.concatenate(
        [np.ascontiguousarray(sh.transpose(2, 1, 0)).reshape(3, R),
         np.ones((1, R), f32)], axis=0)             # [4, R]

    return {
        "im2colT": im2colT.astype(bf16), "beat_bc": beat_bc.astype(bf16),
        "e48": e48, "note0T": note0T.astype(bf16),
        "shiftedT": shiftedT.astype(bf16),
    }


def kernel(**inputs):
    from concourse.bass_utils import run_bass_kernel_spmd

    if "prog" not in _PROGRAM_CACHE:
        _PROGRAM_CACHE["prog"] = _build_program()
    nc = _PROGRAM_CACHE["prog"]

    wmap = _host_prep_weights(inputs)
    note = np.asarray(inputs["note_input"], np.float32)
    beat = np.asarray(inputs["beat_in"], np.float32)
    cond = np.asarray(inputs["condition_notes"], np.float32)

    in_maps = []
    for c in range(N_CORES):
        bs = slice(c * B, (c + 1) * B)
        m = dict(wmap)
        m.update(_host_prep_core(note[bs], beat[bs], cond[bs]))
        in_maps.append(m)

    res = run_bass_kernel_spmd(nc, in_maps, list(range(N_CORES)))
    outs = [res.results[c]["y"].reshape(B, NN, 3) for c in range(N_CORES)]
    return np.concatenate(outs, axis=0).astype(np.float32)



# revision 30
# speedup vs baseline: 1.2142x; 1.2142x over previous
"""DeepJ (TimeAxis + NoteAxis LSTM) Trainium2 kernel.

Data-parallel over 8 NeuronCores: batch 1024 -> 128 per core.

Layout strategy ("everything transposed"):
  activations live as [units, rows] tiles with rows = (note, batch) on the
  free dimension; weights are the stationary (lhsT) matmul operands.  The
  NoteAxis recurrence then needs no per-step transposes: each step's gate
  matmuls consume the previous step's h tiles directly as rhs.

All matmuls run in bfloat16 (PE full rate).  The NoteAxis input projection
(lnf @ nf + lsh @ shifted, bias included) is materialized ahead of the
recurrence into one note-major SBUF tile G0all (col = n*512 + q*128 + b)
by a TimeAxis-style batched pass, then injected into PSUM per step with a
single identity-stationary 512-col matmul; the per-step loop only runs the
three recurrent weight matrices (lhh0, lih1, lhh1) plus the injects.

PSUM accumulation rule (hardware-verified): a start=True matmul clears the
has_written bits of its ENTIRE PSUM bank, so each bank gets exactly one
start=True per accumulation round (the first matmul into it); all other
matmuls use start=False - first touch of a region overwrites via the
cleared bits, later touches accumulate.

The PE executes its queue in order, so the L1 recurrent matmul (lhh1,
which waits on h1) is emitted in na_step AFTER lhh0 - the critical
L0->L0 recurrence never queues behind the L1 chain.
"""

import sys

for _p in ("/opt/trn_rl_repo",):
    if _p not in sys.path:
        sys.path.insert(0, _p)

import numpy as np

# ---- model constants -------------------------------------------------------
N_CORES = 8
B_TOT = 1024
B = B_TOT // N_CORES          # 128 rows per core
NN = 48                       # notes
OCT = 12
R = NN * B                    # 6144 rows, ordered (note, batch)
NBLK = 12                     # row blocks of 512 for the feed-forward stages
BLK = 512

_PROGRAM_CACHE = {}


def _build_program():
    import concourse.tile as tile
    from concourse import bacc, mybir



    f32 = mybir.dt.float32
    bf16 = mybir.dt.bfloat16

    nc = bacc.Bacc(
        "TRN2", target_bir_lowering=False, debug=False, num_devices=N_CORES
    )

    def param(name, shape, dtype=f32):
        return nc.declare_dram_parameter(name, list(shape), dtype, isOutput=False)

    P = {}
    # per-core activations / gathered inputs
    P["im2colT"] = param("im2colT", [75, R], bf16)  # conv patches, (c*25+s, (n,b))
    P["beat_bc"] = param("beat_bc", [16, R], bf16)  # beat_in^T broadcast over n
    P["e48"] = param("e48", [48, R], bf16)          # one-hot(n) broadcast over b
    P["note0T"] = param("note0T", [48, B], bf16)    # note_input[:,:,0]^T
    P["shiftedT"] = param("shiftedT", [4, R], bf16)  # row 3 = ones (bias)
    P["outb_bc"] = param("outb_bc", [128, 3])
    # weights (replicated on every core)
    P["w0comb"] = param("w0comb", [108, 768], bf16)  # folded TA-L0 lhsT
    P["lvic"] = param("lvic", [75, 32], bf16)        # conv lhsT
    P["vicb"] = param("vicb", [32, 1])
    P["lsel"] = param("lsel", [48, 12], bf16)        # chord selection lhsT
    P["w1a"] = param("w1a", [128, 768], bf16)        # TA-L1 lhsT rows 0-127
    P["w1b"] = param("w1b", [128, 768], bf16)        # TA-L1 lhsT rows 128-255
    P["b1cols"] = param("b1cols", [128, 6])        # TA-L1 bias, col per chunk
    P["lnf0"] = param("lnf0", [128, 512], bf16)    # NA-L0 Wih (nf) lhsT
    P["lnf1"] = param("lnf1", [128, 512], bf16)
    P["lsh"] = param("lsh", [4, 512], bf16)        # NA-L0 Wih shifted+bias lhsT
    P["lhh0"] = param("lhh0", [128, 512], bf16)    # NA-L0 Whh lhsT
    P["lih1"] = param("lih1", [128, 512], bf16)    # NA-L1 Wih lhsT
    P["lhh1"] = param("lhh1", [128, 512], bf16)    # NA-L1 Whh lhsT
    P["bias_bc"] = param("bias_bc", [128, 512], bf16)  # NA-L1 bias bcast (g 2x)
    P["ident"] = param("ident", [128, 128], bf16)  # identity (inject lhsT)
    P["outWT"] = param("outWT", [128, 3], bf16)
    P["yout"] = nc.declare_dram_parameter("y", [B, NN * 3], f32, isOutput=True)
    import os as _os
    if _os.environ.get("DEEPJ_DEBUG"):
        for nm, shp, dt in [("d_xt", [108, R], bf16), ("d_h0a", [128, R], bf16),
                            ("d_h0b", [128, R], bf16), ("d_nfa", [128, R], bf16),
                            ("d_nfb", [128, R], bf16), ("d_h1", [128, R], bf16),
                            ("d_g0", [128, 512 * NN], f32),
                            ("d_s0", [128, 512 * NN], f32),
                            ("d_g1", [128, 512 * NN], f32),
                            ("d_h0r", [128, 128 * NN], f32)]:
            P[nm] = nc.declare_dram_parameter(nm, shp, dt, isOutput=True)

    with tile.TileContext(nc) as tc:
        _emit(nc, tc, mybir, P)
    nc.compile()
    return nc


def _emit(nc, tc, mybir, P):
    import os as _os
    from contextlib import ExitStack

    f32 = mybir.dt.float32
    bf16 = mybir.dt.bfloat16
    AF = mybir.ActivationFunctionType
    Alu = mybir.AluOpType

    with ExitStack() as top:
        wpool = top.enter_context(tc.tile_pool(name="weights", bufs=1))
        persist = top.enter_context(tc.tile_pool(name="persist", bufs=1))
        scr = top.enter_context(tc.tile_pool(name="scr", bufs=1))
        nascr = top.enter_context(tc.tile_pool(name="nascr", bufs=2))
        h0ring = top.enter_context(tc.tile_pool(name="h0ring", bufs=3))
        cpool = top.enter_context(tc.tile_pool(name="cstate", bufs=2))
        # PSUM budget (8 banks): pio [128,1024] x2 bufs = 4, ps0 1, ps1 1,
        # pout (g0/conv/chord/outproj) [128,512] x2 bufs = 2.
        pta = top.enter_context(tc.tile_pool(name="pta", bufs=2, space="PSUM"))
        pna0 = top.enter_context(tc.tile_pool(name="pna0", bufs=1, space="PSUM"))
        pna1 = top.enter_context(tc.tile_pool(name="pna1", bufs=1, space="PSUM"))
        pout = top.enter_context(tc.tile_pool(name="pout", bufs=2, space="PSUM"))

        def wload(name, shape, dtype=f32):
            t = wpool.tile(list(shape), dtype, tag=name, name=name)
            nc.sync.dma_start(t[:], P[name][:])
            return t

        # persistent activations (declare first; DMA issue order below is
        # the startup-critical-path order)
        xt = persist.tile([108, R], bf16, tag="xt")
        h0T = [persist.tile([128, R], bf16, tag=f"h0T{i}", name=f"h0T{i}")
               for i in range(2)]
        nfT = [persist.tile([128, R], bf16, tag=f"nfT{i}", name=f"nfT{i}")
               for i in range(2)]
        # NA-L0 input projection, note-major: col = n*512 + q*128 + b
        G0all = persist.tile([128, 4 * R], bf16, tag="G0all")
        h1All = persist.tile([128, R], bf16, tag="h1All")
        imT = persist.tile([75, R], bf16, tag="imT")

        # startup-critical loads first: conv + chord + TA-L0 inputs
        nc.sync.dma_start(imT[:, 0:2 * BLK], P["im2colT"][:, 0:2 * BLK])
        lvic_t = wload("lvic", [75, 32], bf16)
        vicb_t = wload("vicb", [32, 1])
        lsel_t = wload("lsel", [48, 12], bf16)
        w0comb_t = wload("w0comb", [108, 768], bf16)
        n0_t = scr.tile([48, B], bf16, tag="note0T")
        nc.sync.dma_start(n0_t[:], P["note0T"][:])
        nc.sync.dma_start(xt[32:48, :], P["beat_bc"][:])
        nc.sync.dma_start(xt[48:96, :], P["e48"][:])
        nc.sync.dma_start(imT[:, 2 * BLK:], P["im2colT"][:, 2 * BLK:])
        w1a_t = wload("w1a", [128, 768], bf16)
        w1b_t = wload("w1b", [128, 768], bf16)
        b1cols_t = wload("b1cols", [128, 6])
        ident_t = wload("ident", [128, 128], bf16)
        lnf0_t = wload("lnf0", [128, 512], bf16)
        lnf1_t = wload("lnf1", [128, 512], bf16)
        lsh_t = wload("lsh", [4, 512], bf16)
        shT_t = wload("shiftedT", [4, R], bf16)
        lhh0_t = wload("lhh0", [128, 512], bf16)
        lih1_t = wload("lih1", [128, 512], bf16)
        lhh1_t = wload("lhh1", [128, 512], bf16)
        bias_bc_t = wload("bias_bc", [128, 512], bf16)
        outWT_t = wload("outWT", [128, 3], bf16)
        outb_t = wload("outb_bc", [128, 3])

        # ---- one-time XT rows: chord ------------------------------------
        cps = pout.tile([32, BLK], f32, tag="g0ps", name="cps")
        nc.tensor.matmul(cps[0:12, 0:B], lsel_t[:], n0_t[:])
        chT = scr.tile([12, B], bf16, tag="chT")
        nc.vector.tensor_copy(chT[:], cps[0:12, 0:B])
        for n in range(NN):
            nc.sync.dma_start(xt[96:108, n * B:(n + 1) * B], chT[:])

        # ---- TA block emitters (chunked for fine-grained weaving) -------
        def ta_conv(blk):
            sl = slice(blk * BLK, (blk + 1) * BLK)
            vps = pout.tile([32, BLK], f32, tag="g0ps", name="vps")
            nc.tensor.matmul(vps[:], lvic_t[:], imT[:, sl])
            nc.scalar.activation(xt[0:32, sl], vps[:], AF.Tanh,
                                 bias=vicb_t[:, 0:1])

        def ta_l0_chunks(blk):
            """TA layer 0 for one 512-row block as 4 weavable chunks."""
            sl = slice(blk * BLK, (blk + 1) * BLK)
            st = {}

            # All activation / DVE / GPSIMD filler ops are emitted in
            # ~512-col pieces: the static schedule interleaves them with the
            # NA chain ops, and a chain op never waits more than one small
            # piece for its engine.
            def act2(out, in_, func, n=2):
                w = 2 * BLK // n
                for j in range(n):
                    js = slice(j * w, (j + 1) * w)
                    nc.scalar.activation(out[:, js], in_[:, js], func)

            def c0():
                x = xt[:, sl]
                io0 = pta.tile([128, 2 * BLK], f32, tag="pio", name="io0")
                nc.tensor.matmul(io0[:, 0:BLK], w0comb_t[:, 0:128], x)
                nc.tensor.matmul(io0[:, BLK:2 * BLK], w0comb_t[:, 512:640], x)
                s0 = scr.tile([128, 2 * BLK], f32, tag="sio0")
                act2(s0, io0, AF.Sigmoid)
                st["s0"] = s0

            def c1():
                x = xt[:, sl]
                io1 = pta.tile([128, 2 * BLK], f32, tag="pio", name="io1")
                nc.tensor.matmul(io1[:, 0:BLK], w0comb_t[:, 128:256], x)
                nc.tensor.matmul(io1[:, BLK:2 * BLK], w0comb_t[:, 640:768], x)
                s1 = scr.tile([128, 2 * BLK], f32, tag="sio1")
                act2(s1, io1, AF.Sigmoid)
                st["s1"] = s1

            def c2():
                x = xt[:, sl]
                gg = pta.tile([128, 2 * BLK], f32, tag="pio", name="gg")
                nc.tensor.matmul(gg[:, 0:BLK], w0comb_t[:, 256:384], x)
                nc.tensor.matmul(gg[:, BLK:2 * BLK], w0comb_t[:, 384:512], x)
                tg = scr.tile([128, 2 * BLK], f32, tag="tg")
                act2(tg, gg, AF.Tanh)
                cc = scr.tile([128, 2 * BLK], f32, tag="c2")
                for j in range(4):
                    js = slice(j * 256, (j + 1) * 256)
                    ss = st["s0"] if j < 2 else st["s1"]
                    nc.gpsimd.tensor_tensor(cc[:, js], ss[:, (j % 2) * 256:
                                            (j % 2) * 256 + 256],
                                            tg[:, js], Alu.mult)
                st["cc"] = cc

            def c3():
                tc2 = scr.tile([128, 2 * BLK], f32, tag="tc2")
                act2(tc2, st["cc"], AF.Tanh)
                for j in range(2):
                    js = slice(j * 256, (j + 1) * 256)
                    nc.vector.tensor_tensor(h0T[0][:, sl][:, js],
                                            st["s0"][:, BLK:2 * BLK][:, js],
                                            tc2[:, 0:BLK][:, js], Alu.mult)
                    nc.vector.tensor_tensor(h0T[1][:, sl][:, js],
                                            st["s1"][:, BLK:2 * BLK][:, js],
                                            tc2[:, BLK:2 * BLK][:, js],
                                            Alu.mult)

            return [c0, c1, c2, c3]

        def _l1_bank(sl, ps, cols, q):
            """One TA-L1 psum bank: w1a/w1b for u-chunk q (bias rides on
            the activation's per-partition bias operand, not a matmul)."""
            qs = slice(q * 128, (q + 1) * 128)
            nc.tensor.matmul(ps[:, cols], w1a_t[:, qs], h0T[0][:, sl],
                             start=True, stop=False, skip_group_check=True)
            nc.tensor.matmul(ps[:, cols], w1b_t[:, qs], h0T[1][:, sl],
                             start=False, stop=True, skip_group_check=True)

        def ta_l1_chunks(blk):
            """TA layer 1 for one 512-row block as 4 weavable chunks."""
            sl = slice(blk * BLK, (blk + 1) * BLK)
            st = {}

            def c0():
                io0 = pta.tile([128, 2 * BLK], f32, tag="pio", name="l1io0")
                _l1_bank(sl, io0, slice(0, BLK), 0)          # i half0
                _l1_bank(sl, io0, slice(BLK, 2 * BLK), 4)    # o half0
                s0 = scr.tile([128, 2 * BLK], f32, tag="sio0", name="bsio0")
                nc.scalar.activation(s0[:, 0:BLK], io0[:, 0:BLK], AF.Sigmoid,
                                     bias=b1cols_t[:, 0:1])
                nc.scalar.activation(s0[:, BLK:2 * BLK], io0[:, BLK:2 * BLK],
                                     AF.Sigmoid, bias=b1cols_t[:, 4:5])
                st["s0"] = s0

            def c1():
                io1 = pta.tile([128, 2 * BLK], f32, tag="pio", name="l1io1")
                _l1_bank(sl, io1, slice(0, BLK), 1)          # i half1
                _l1_bank(sl, io1, slice(BLK, 2 * BLK), 5)    # o half1
                s1 = scr.tile([128, 2 * BLK], f32, tag="sio1", name="bsio1")
                nc.scalar.activation(s1[:, 0:BLK], io1[:, 0:BLK], AF.Sigmoid,
                                     bias=b1cols_t[:, 1:2])
                nc.scalar.activation(s1[:, BLK:2 * BLK], io1[:, BLK:2 * BLK],
                                     AF.Sigmoid, bias=b1cols_t[:, 5:6])
                st["s1"] = s1

            def c2():
                gg = pta.tile([128, 2 * BLK], f32, tag="pio", name="l1gg")
                _l1_bank(sl, gg, slice(0, BLK), 2)           # g half0
                _l1_bank(sl, gg, slice(BLK, 2 * BLK), 3)     # g half1
                tg = scr.tile([128, 2 * BLK], f32, tag="tg", name="btg")
                nc.scalar.activation(tg[:, 0:BLK], gg[:, 0:BLK], AF.Tanh,
                                     bias=b1cols_t[:, 2:3])
                nc.scalar.activation(tg[:, BLK:2 * BLK], gg[:, BLK:2 * BLK],
                                     AF.Tanh, bias=b1cols_t[:, 3:4])
                cc = scr.tile([128, 2 * BLK], f32, tag="c2", name="bc2")
                for j in range(4):
                    js = slice(j * 256, (j + 1) * 256)
                    ss = st["s0"] if j < 2 else st["s1"]
                    nc.gpsimd.tensor_tensor(cc[:, js], ss[:, (j % 2) * 256:
                                            (j % 2) * 256 + 256],
                                            tg[:, js], Alu.mult)
                st["cc"] = cc

            def c3():
                tc2 = scr.tile([128, 2 * BLK], f32, tag="tc2", name="btc2")
                for j in range(2):
                    js = slice(j * BLK, (j + 1) * BLK)
                    nc.scalar.activation(tc2[:, js], st["cc"][:, js], AF.Tanh)
                for j in range(2):
                    js = slice(j * 256, (j + 1) * 256)
                    nc.vector.tensor_tensor(nfT[0][:, sl][:, js],
                                            st["s0"][:, BLK:2 * BLK][:, js],
                                            tc2[:, 0:BLK][:, js], Alu.mult)
                    nc.vector.tensor_tensor(nfT[1][:, sl][:, js],
                                            st["s1"][:, BLK:2 * BLK][:, js],
                                            tc2[:, BLK:2 * BLK][:, js],
                                            Alu.mult)

            return [c0, c1, c2, c3]

        G0v = G0all[:].rearrange("p (n q b) -> p n q b", q=4, b=B)

        def g0_chunk(blk, q):
            """NA-L0 input projection for gate q over 4 notes (512 rows)."""
            sl = slice(blk * BLK, (blk + 1) * BLK)
            qs = slice(q * 128, (q + 1) * 128)
            gps = pout.tile([128, BLK], f32, tag="g0ps", name="g0ps")
            nc.tensor.matmul(gps[:], lnf0_t[:, qs], nfT[0][:, sl],
                             start=True, stop=False, skip_group_check=True)
            nc.tensor.matmul(gps[:], lnf1_t[:, qs], nfT[1][:, sl],
                             start=False, stop=False, skip_group_check=True)
            nc.tensor.matmul(gps[:], lsh_t[:, qs], shT_t[:, sl],
                             start=False, stop=True, skip_group_check=True)
            # GPSIMD cannot read PSUM, so this cast stays on the DVE
            # (two 2-note pieces to bound NA-chain queue delay)
            g3 = gps[:].rearrange("p (n b) -> p n b", b=B)
            for j in range(2):
                nc.vector.tensor_copy(
                    G0v[:, 4 * blk + 2 * j:4 * blk + 2 * j + 2, q, :],
                    g3[:, 2 * j:2 * j + 2, :])

        # ---- NoteAxis pair-emission pipeline ---------------------------
        # pair_emit(k) emits L0(k) and L1(k-1) with the ops interleaved so
        # the L0->L0 recurrence (lhh0 -> sigma0 -> DVE -> tanh -> h0r) owns
        # the front of every engine queue; L1(k-1) trails.  ps1(k-1) gets
        # lih1 BEFORE lhh1 (h1(k-2) is long ready), so the PE never stalls
        # on a fresh h1.  t2 = f*c_prev runs on GPSIMD, off the DVE path.
        ps0_state = {}
        ps1_state = {}
        h0_ring = {}
        c_prev = [None, None]
        pso_box = [None]

        def na_open0(k):
            ps0 = pna0.tile([128, 512], f32, tag="na0", name="ps0")
            nc.tensor.matmul(ps0[:], ident_t[:],
                             G0all[:, k * 512:(k + 1) * 512],
                             start=True, stop=(k == 0),
                             skip_group_check=True)
            ps0_state[k] = ps0

        def na_open1(k):
            """ps1 bias inject for L1(k); lih1/lhh1 accumulate next pair."""
            ps1 = pna1.tile([128, 512], f32, tag="na1", name="ps1")
            nc.tensor.matmul(ps1[:, 0:512], ident_t[:], bias_bc_t[:],
                             start=True, stop=False, skip_group_check=True)
            ps1_state[k] = ps1

        def _nl_front(s, li):
            """sigmoid -> gt/t1/(t2)/c chain for layer li; returns c_new.

            The cell state is stored HALVED (c' = c/2): with the host-side
            sigma-trick (g rows doubled), tanh(g) = 2*sigmoid(2g)-1, so
              c' = si*(sg - 0.5) + sf*c'_prev
            needs only a 1-ALU tensor_scalar, and tanh(c) = tanh(2*c') rides
            the activation's free scale operand downstream.
            """
            tag = f"L{li}"
            si, sf, sg, so = (s[:, 128 * j:128 * (j + 1)] for j in range(4))
            gt = nascr.tile([128, 128], f32, tag=f"{tag}gt")
            nc.vector.tensor_scalar(gt[:], sg, -0.5, None, Alu.add)
            # (1-ALU op; the former 2*s-1 needed mult+add at half DVE rate)
            c_new = cpool.tile([128, 128], f32, tag=f"{tag}c")
            if c_prev[li] is None:
                nc.vector.tensor_tensor(c_new[:], si, gt[:], Alu.mult)
            else:
                t2 = nascr.tile([128, 128], f32, tag=f"{tag}t2")
                nc.gpsimd.tensor_tensor(t2[:], sf, c_prev[li][:], Alu.mult)
                t1 = nascr.tile([128, 128], f32, tag=f"{tag}t1")
                nc.vector.tensor_tensor(t1[:], si, gt[:], Alu.mult)
                # L0's add is on the h0r critical path (fast DVE); L1's is
                # not, so it rides the idler GPSIMD engine.
                eng = nc.vector if li == 0 else nc.gpsimd
                eng.tensor_tensor(c_new[:], t1[:], t2[:], Alu.add)
            c_prev[li] = c_new
            return c_new

        def pair_emit(k):
            # The NA recurrence is the serial critical path of the whole
            # kernel: give its ops priority 0 so the list scheduler runs
            # them the moment their deps resolve, with TA/G0 work as the
            # natural filler on every engine.
            with tc.high_priority():
                _pair_emit(k)

        def _pair_emit(k):
            has0 = k < NN
            has1 = k >= 1
            # PE front: the two matmul groups gated on h0r(k-1)
            if has0:
                ps0 = ps0_state.pop(k)
                if k > 0:
                    h0p = h0_ring[k - 1]
                    for q in range(4):
                        qs = slice(q * 128, (q + 1) * 128)
                        nc.tensor.matmul(ps0[:, qs], lhh0_t[:, qs], h0p[:],
                                         start=False, stop=(q == 3),
                                         skip_group_check=True)
            if has1:
                ps1 = ps1_state.pop(k - 1)
                h0p = h0_ring.pop(k - 1)
                for q in range(4):
                    qs = slice(q * 128, (q + 1) * 128)
                    nc.tensor.matmul(ps1[:, qs], lih1_t[:, qs], h0p[:],
                                     start=False, stop=(q == 3 and k < 2),
                                     skip_group_check=True)
                if k >= 2:
                    pns = slice((k - 2) * B, (k - 1) * B)
                    for q in range(4):
                        qs = slice(q * 128, (q + 1) * 128)
                        nc.tensor.matmul(ps1[:, qs], lhh1_t[:, qs],
                                         h1All[:, pns], start=False,
                                         stop=(q == 3), skip_group_check=True)
            # scalar front: sigma0 before sigma1
            s0 = s1 = None
            if has0:
                s0 = nascr.tile([128, 512], f32, tag="L0s")
                nc.scalar.activation(s0[:], ps0[:], AF.Sigmoid)
            if has1:
                s1 = nascr.tile([128, 512], f32, tag="L1s")
                nc.scalar.activation(s1[:], ps1[:], AF.Sigmoid)
            # DVE chains: L0 first, then L1
            if has0:
                c0n = _nl_front(s0[:], 0)
                tc0 = nascr.tile([128, 128], f32, tag="L0tc")
                nc.scalar.activation(tc0[:], c0n[:], AF.Tanh, scale=2.0)
                h0r = h0ring.tile([128, B], bf16, tag="h0r", name="h0r")
                h0_ring[k] = h0r
                nc.vector.tensor_tensor(h0r[:], s0[:, 384:512], tc0[:],
                                        Alu.mult)
            if has1:
                c1n = _nl_front(s1[:], 1)
                tc1 = nascr.tile([128, 128], f32, tag="L1tc")
                nc.scalar.activation(tc1[:], c1n[:], AF.Tanh, scale=2.0)
                nc.vector.tensor_tensor(h1All[:, (k - 1) * B:k * B],
                                        s1[:, 384:512], tc1[:], Alu.mult)
            # openers
            if has0 and k + 1 < NN:
                na_open0(k + 1)
            if has0:
                na_open1(k)

        def outproj(n):
            nc.tensor.matmul(pso_box[0][:, 3 * n:3 * n + 3],
                             h1All[:, n * B:(n + 1) * B], outWT_t[:],
                             skip_group_check=True)

        # ---- interleaved pipeline: fine-grained TA/NA weave ------------
        for blk in range(NBLK):
            slots = [lambda b=blk: ta_conv(b)]
            slots += ta_l0_chunks(blk)
            if blk >= 1:
                slots.append(lambda b=blk: g0_chunk(b - 1, 0))
                slots.append(lambda b=blk: g0_chunk(b - 1, 1))
            slots += ta_l1_chunks(blk)
            if blk >= 1:
                slots.append(lambda b=blk: g0_chunk(b - 1, 2))
                slots.append(lambda b=blk: g0_chunk(b - 1, 3))
            if blk > 1:
                pairs = [4 * (blk - 2) + j for j in range(4)]
                # spread the 4 pair_emits across the 13 slots
                order = []
                pi = 0
                for i, sfn in enumerate(slots):
                    order.append(sfn)
                    if i in (2, 5, 8, 12) and pi < 4:
                        order.append(lambda p=pairs[pi]: pair_emit(p))
                        pi += 1
                for fn in order:
                    fn()
            else:
                for fn in slots:
                    fn()
            if blk == 1:
                na_open0(0)
        for q in range(4):
            g0_chunk(NBLK - 1, q)
        pso_box[0] = pout.tile([128, BLK], f32, tag="g0ps", name="pso")
        pso = pso_box[0]
        for j in range(9):
            pair_emit(40 + j)
            for n in range(5 * j, min(5 * j + 5, 40)):  # backlog notes 0..39
                outproj(n)
        for n in range(40, NN):
            outproj(n)

        # ---- output sigmoid + store ------------------------------------
        out_sb = scr.tile([128, NN * 3], f32, tag="osb")
        ps3d = pso[:, 0:NN * 3].rearrange("p (n c) -> p n c", c=3)
        o3d = out_sb[:].rearrange("p (n c) -> p n c", c=3)
        nc.scalar.activation(o3d[:, :, 0], ps3d[:, :, 0], AF.Sigmoid,
                             bias=outb_t[:, 0:1])
        nc.scalar.activation(o3d[:, :, 1], ps3d[:, :, 1], AF.Sigmoid,
                             bias=outb_t[:, 1:2])
        nc.scalar.activation(o3d[:, :, 2], ps3d[:, :, 2], AF.Identity,
                             bias=outb_t[:, 2:3])
        nc.sync.dma_start(P["yout"][:], out_sb[:])


# --------------------------------------------------------------------------
# host side
# --------------------------------------------------------------------------

def _host_prep_weights(inp):
    import ml_dtypes

    f32 = np.float32
    bf16 = ml_dtypes.bfloat16

    W0 = np.asarray(inp["ta_Wih0"], f32)          # [1024, 73]
    sel = np.r_[0:256, 512:768, 768:1024]
    W0s = W0[sel]                                  # [768, 73] rows i,g,o
    b0s = (np.asarray(inp["ta_bih0"], f32) + np.asarray(inp["ta_bhh0"], f32))[sel]

    n = np.arange(NN)
    const_feat = np.zeros((13, NN), f32)
    const_feat[0] = n / NN
    const_feat[1 + (n % OCT), n] = 1.0

    beat_W = np.asarray(inp["beat_W"], f32)        # [16, 16]
    beat_b = np.asarray(inp["beat_b"], f32)
    gn = (W0s[:, 0:13] @ const_feat
          + (b0s + W0s[:, 13:29] @ beat_b)[:, None])        # [768, 48]
    Wbeat = W0s[:, 13:29] @ beat_W                 # [768, 16]
    Wvic = W0s[:, 29:61]                           # [768, 32]
    Wchord = W0s[:, 61:73]                         # [768, 12]
    w0comb = np.concatenate(
        [Wvic.T, Wbeat.T, gn.T, Wchord.T], axis=0
    ).astype(bf16)                                 # [108, 768]

    vic_W = np.asarray(inp["vic_W"], f32)          # [32, 3, 25]
    lvic = vic_W.reshape(32, 75).T.copy().astype(bf16)  # [75, 32] rows (c*25+s)
    vicb = np.asarray(inp["vic_b"], f32).reshape(32, 1)

    lsel = np.zeros((48, 12), f32)
    lsel[np.arange(48), np.arange(48) // 4] = 0.25
    lsel = lsel.astype(bf16)

    W1 = np.asarray(inp["ta_Wih1"], f32)[sel]      # [768, 256]
    b1s = (np.asarray(inp["ta_bih1"], f32) + np.asarray(inp["ta_bhh1"], f32))[sel]
    w1T = W1.T.astype(bf16)                        # [256, 768]
    b1cols = np.ascontiguousarray(b1s.reshape(6, 128).T)   # [128, 6] f32

    # sigma-trick: tanh(g) = 2*sigmoid(2g)-1, so double every g-gate row
    # (cols 256:384 of the transposed layouts) including the bias.
    def dbl_g(wT):
        wT = wT.copy()
        wT[:, 256:384] *= 2.0
        return wT

    naW0 = np.asarray(inp["na_Wih0"], f32)         # [512, 259]
    lnf = dbl_g(naW0[:, 0:256].T).astype(bf16)     # [256, 512]
    nb0 = (np.asarray(inp["na_bih0"], f32) + np.asarray(inp["na_bhh0"], f32))
    lsh = np.concatenate([naW0[:, 256:259].T, nb0[None, :]], axis=0)
    lsh = dbl_g(lsh).astype(bf16)                  # [4, 512]
    lhh0 = dbl_g(np.asarray(inp["na_Whh0"], f32).T).astype(bf16)
    lih1 = dbl_g(np.asarray(inp["na_Wih1"], f32).T).astype(bf16)
    lhh1 = dbl_g(np.asarray(inp["na_Whh1"], f32).T).astype(bf16)
    nb1 = (np.asarray(inp["na_bih1"], f32) + np.asarray(inp["na_bhh1"], f32))
    nb1d = nb1.copy()
    nb1d[256:384] *= 2.0                           # g-gate bias doubled
    # bias_bc[u, q*128 + b] = nb1d[q*128 + u], broadcast over batch b
    bias_bc = np.ascontiguousarray(
        np.broadcast_to(nb1d.reshape(4, 128).T[:, :, None], (128, 4, 128))
    ).reshape(128, 512).astype(bf16)

    outWT = np.asarray(inp["out_W"], f32).T.astype(bf16)     # [128, 3]
    outb_bc = np.broadcast_to(
        np.asarray(inp["out_b"], f32), (128, 3)
    ).copy()

    return {
        "w0comb": w0comb, "lvic": lvic, "vicb": vicb, "lsel": lsel,
        "w1a": w1T[0:128].copy(), "w1b": w1T[128:256].copy(), "b1cols": b1cols,
        "lnf0": lnf[0:128].copy(), "lnf1": lnf[128:256].copy(),
        "lsh": lsh, "lhh0": lhh0,
        "lih1": lih1, "lhh1": lhh1, "bias_bc": bias_bc,
        "ident": np.eye(128, dtype=bf16),
        "outWT": outWT, "outb_bc": outb_bc,
    }


def _host_prep_core(note, beat, cond):
    """Per-core input gathering (indexing only). note [B,48,3] etc."""
    import ml_dtypes

    f32 = np.float32
    bf16 = ml_dtypes.bfloat16
    pn = np.zeros((B, 72, 3), f32)
    pn[:, 12:60, :] = note
    # im2colT[(c*25+s), (n, b)] = pn[b, n+s, c]
    win = np.stack([pn[:, s:s + 48, :] for s in range(25)], axis=0)  # [25,B,48,3]
    im2colT = np.ascontiguousarray(win.transpose(3, 0, 2, 1)).reshape(75, R)

    beat_bc = np.ascontiguousarray(
        np.broadcast_to(beat.T[:, None, :], (16, NN, B))
    ).reshape(16, R)
    e48 = np.repeat(np.eye(48, dtype=bf16), B, axis=1)       # [48, R]
    note0T = np.ascontiguousarray(note[:, :, 0].T)           # [48, B]

    sh = np.zeros((B, NN, 3), f32)
    sh[:, 1:, :] = cond[:, :-1, :]
    shiftedT = np.concatenate(
        [np.ascontiguousarray(sh.transpose(2, 1, 0)).reshape(3, R),
         np.ones((1, R), f32)], axis=0)             # [4, R]

    return {
        "im2colT": im2colT.astype(bf16), "beat_bc": beat_bc.astype(bf16),
        "e48": e48, "note0T": note0T.astype(bf16),
        "shiftedT": shiftedT.astype(bf16),
    }


def kernel(**inputs):
    from concourse.bass_utils import run_bass_kernel_spmd

    if "prog" not in _PROGRAM_CACHE:
        _PROGRAM_CACHE["prog"] = _build_program()
    nc = _PROGRAM_CACHE["prog"]

    wmap = _host_prep_weights(inputs)
    note = np.asarray(inputs["note_input"], np.float32)
    beat = np.asarray(inputs["beat_in"], np.float32)
    cond = np.asarray(inputs["condition_notes"], np.float32)

    in_maps = []
    for c in range(N_CORES):
        bs = slice(c * B, (c + 1) * B)
        m = dict(wmap)
        m.update(_host_prep_core(note[bs], beat[bs], cond[bs]))
        in_maps.append(m)

    res = run_bass_kernel_spmd(nc, in_maps, list(range(N_CORES)))
    outs = [res.results[c]["y"].reshape(B, NN, 3) for c in range(N_CORES)]
    return np.concatenate(outs, axis=0).astype(np.float32)

